# revision 28
# baseline (speedup 1.0000x reference)
"""Trainium2 Bass kernel for the sparse_attention nn problem.

Math (per sample n, all shapes per sample):
  t1_d = x shifted by 2*(d-1) rows (zero pad), d in {0,1,2}
  t2_d = p2w_d * t1_d
  t3_d = x shifted by 2*(d-1) cols (zero pad)
  t4   = roll(x, 1, rows);  t6 = t4 + x
  t7[k=(d,c1), c] = sum_p t2_d[c1,p] * t6[c,p] / 56
  t8full = W'' @ t4 with W''[c,c'] = conv_w[c%4,c'] * p5w[c']   (t9 = t8full*x)
  t10[c,p] = sum_{d,c1} t7[(d,c1),c] * t3_d[c1,p] / sqrt(384)
  out = t9 + t10
k-order is permuted (d-major) consistently in t7/t10 -> result unchanged.
Scales folded into p2w on host: p2w' = p2w / (56*sqrt(384)).

Sharding: pure data parallel over batch (4 samples per core, 8 cores).

v2 design notes:
 - x is cast to bf16 on the host; output written bf16 (halves DMA traffic).
 - x^T built on PE (28 transposes of (128x112) into 4 one-bank PSUM groups),
   evacuated to SBUF by ACT.
 - t2^T = p2w^T * x^T(d-shifted): 3 DVE ops (free-dim chunk shifts).
 - t6 = x + roll(x) built on DVE in natural layout (free-dim shifts),
   transposed on PE like x.
 - t7 accumulated directly in the [(d,c1), c] orientation (3 sequential
   PSUM col-groups, 84 narrow matmuls) -> single ACT evacuation, no
   extra transposes.
 - t3 col shifts are folded into the t10 matmuls by reading a zero-halo
   padded copy of x (xpad, row stride 60) -- zero materialization of t3.
 - output stage per 448-col chunk: t8 matmul; t9 = x*t8 on DVE (from PSUM
   or via an ACT-staged bf16 copy, alternating to balance engines); t10 as
   a standard 3-matmul accumulation group; ACT evacuates t10 as bf16 and
   DVE adds t9 + t10 into the bf16 output tile.
"""

import math
import numpy as np
import ml_dtypes

N, C, H, W, G = 32, 128, 56, 56, 32
HW = H * W                # 3136
NCORES = 8
NS = N // NCORES          # 4 samples per core
PCH = 2 * W               # 112 = p-chunk (2 image rows)
NCH = HW // PCH           # 28 chunks
HP = 56                   # half-chunk (1 image row) partition offset
NG = 4                    # psum transpose groups
GB = NCH // NG            # 7 blocks per group (1 bank)
FCH = 448                 # t10/t8 free chunk (8 rows)
NF = HW // FCH            # 7
SCALE = 1.0 / (56.0 * math.sqrt(384.0))
BF16NP = ml_dtypes.bfloat16

_CACHE = {}


def _body(tc, bass, mybir, xs, p2wt, w2t, ident, out, BF, F32, ctx, repeat=1, loop=1):
    nc = tc.nc
    mult = mybir.AluOpType.mult
    addop = mybir.AluOpType.add

    consts = ctx.enter_context(tc.tile_pool(name="consts", bufs=1))
    p_xbf = ctx.enter_context(tc.tile_pool(name="xbf", bufs=3))
    p_xtsb = ctx.enter_context(tc.tile_pool(name="xtsb", bufs=3))
    p_t6n = ctx.enter_context(tc.tile_pool(name="t6nn", bufs=2))
    p_t6t = ctx.enter_context(tc.tile_pool(name="t6t", bufs=3))
    p_t7sb = ctx.enter_context(tc.tile_pool(name="t7sb", bufs=3))
    p_out = ctx.enter_context(tc.tile_pool(name="outp", bufs=3))
    p_fsb = ctx.enter_context(tc.tile_pool(name="fsb", bufs=3))
    p_t9 = ctx.enter_context(tc.tile_pool(name="t9p", bufs=7))
    ps_tr = ctx.enter_context(tc.tile_pool(name="pstr", bufs=3, space="PSUM"))
    ps_t7 = ctx.enter_context(tc.tile_pool(name="pst7", bufs=2, space="PSUM"))
    ps_f = ctx.enter_context(tc.tile_pool(name="psf", bufs=3, space="PSUM"))

    p2wt_t = consts.tile([PCH, NCH * 3 * C], BF)
    nc.sync.dma_start(p2wt_t[:], p2wt)
    w2t_t = consts.tile([C, C], BF)
    nc.sync.dma_start(w2t_t[:], w2t)
    id_t = consts.tile([C, C], BF)
    nc.sync.dma_start(id_t[:], ident)

    # two persistent t2t buffers (alternate across samples); their boundary
    # blocks (j=0,d=0) and (j=27,d=2) are zero forever (shifted-x zero pad).
    t2ts = []
    for k in range(2):
        t2t = consts.tile([PCH, NCH * 3 * C], BF, tag=f"t2t{k}")
        nc.gpsimd.memset(t2t[:, 0:C], 0.0)
        nc.gpsimd.memset(t2t[:, (NCH * 3 - 1) * C : NCH * 3 * C], 0.0)
        t2ts.append(t2t)
    pwr = p2wt_t[:].rearrange("p (j x) -> p j x", x=3 * C)

    # x with a 2-col zero halo per row (row stride 60) for the t3 col shifts
    WP = W + 4
    xpads = []
    for k in range(2):
        xp = consts.tile([C, H * WP], BF, tag=f"xpad{k}")
        xpr = xp[:].rearrange("c (h w) -> c h w", w=WP)
        nc.gpsimd.memset(xpr[:, :, 0:2], 0.0)
        nc.gpsimd.memset(xpr[:, :, W + 2 : WP], 0.0)
        xpads.append(xp)

    def emit_samples():
     for si, s in enumerate([s for _ in range(repeat) for s in range(NS)]):
         xbf = p_xbf.tile([C, HW], BF, tag="xbf")
         nc.sync.dma_start(xbf[:], xs[s])

         # padded copy for the t10 col-shifted moving operands
         xpad = xpads[si % 2]
         xpr = xpad[:].rearrange("c (h w) -> c h w", w=WP)
         nc.gpsimd.tensor_copy(
             xpr[:, :, 2 : W + 2],
             xbf[:].rearrange("c (h w) -> c h w", w=W),
         )

         # --- transpose x into PSUM: 4 groups of 7 chunks (1 bank each) ---
         xt_sb = p_xtsb.tile([PCH, NCH * C], BF, tag="xtsb")
         for g in range(NG):
             xtg = ps_tr.tile([PCH, GB * C], BF, tag="tr")
             for r in range(GB):
                 j = g * GB + r
                 nc.tensor.transpose(
                     xtg[:, r * C : (r + 1) * C],
                     xbf[:, j * PCH : (j + 1) * PCH],
                     id_t[:],
                 )
             nc.scalar.copy(
                 xt_sb[:, g * GB * C : (g + 1) * GB * C], xtg[:, 0 : GB * C]
             )

         # --- t2^T = p2w^T * x^T(shifted) --- 3 DVE ops via chunk shifts
         t2t = t2ts[si % 2]
         t2r = t2t[:].rearrange("p (j x) -> p j x", x=3 * C)
         for d in range(3):
             j0 = 1 if d == 0 else 0
             j1 = NCH - 1 if d == 2 else NCH
             nc.vector.tensor_tensor(
                 out=t2r[:, j0:j1, d * C : (d + 1) * C],
                 in0=pwr[:, j0:j1, d * C : (d + 1) * C],
                 in1=xt_sb[:, (j0 + d - 1) * C : (j1 + d - 1) * C].rearrange(
                     "p (j c) -> p j c", c=C
                 ),
                 op=mult,
             )

         # --- t6 = x + roll(x, 1 row) in natural layout, then transpose ---
         t6n = p_t6n.tile([C, HW], BF, tag="t6n")
         nc.vector.tensor_tensor(
             out=t6n[:, W:HW], in0=xbf[:, W:HW], in1=xbf[:, 0 : HW - W], op=addop
         )
         nc.vector.tensor_tensor(
             out=t6n[:, 0:W], in0=xbf[:, 0:W], in1=xbf[:, HW - W : HW], op=addop
         )
         t6t = p_t6t.tile([PCH, NCH * C], BF, tag="t6t")
         for g in range(NG):
             t6g = ps_tr.tile([PCH, GB * C], BF, tag="tr")
             for r in range(GB):
                 j = g * GB + r
                 nc.tensor.transpose(
                     t6g[:, r * C : (r + 1) * C],
                     t6n[:, j * PCH : (j + 1) * PCH],
                     id_t[:],
                 )
             nc.scalar.copy(
                 t6t[:, g * GB * C : (g + 1) * GB * C], t6g[:, 0 : GB * C]
             )

         # --- t8 matmuls hoisted: only need xbf, fill PE gaps early ---
         t8list = []
         for f in range(NF):
             lo = f * FCH
             t8ps = ps_f.tile([C, FCH], F32, tag="fps")
             if f == 0:
                 nc.tensor.matmul(
                     t8ps[:, 0:W], w2t_t[:], xbf[:, HW - W : HW],
                     start=True, stop=False, skip_group_check=True,
                 )
                 nc.tensor.matmul(
                     t8ps[:, W:FCH], w2t_t[:], xbf[:, 0 : FCH - W],
                     start=True, stop=True, skip_group_check=True,
                 )
             else:
                 nc.tensor.matmul(
                     t8ps[:], w2t_t[:], xbf[:, lo - W : lo + FCH - W],
                     start=True, stop=True,
                 )
             t9sb = p_t9.tile([C, FCH], BF, tag="t9sb")
             if f % 2 == 0:
                 t8sb = p_fsb.tile([C, FCH], BF, tag="t8sb")
                 nc.scalar.copy(t8sb[:], t8ps[:])
                 nc.vector.tensor_tensor(
                     out=t9sb[:], in0=xbf[:, lo : lo + FCH], in1=t8sb[:], op=mult
                 )
             else:
                 nc.vector.tensor_tensor(
                     out=t9sb[:], in0=xbf[:, lo : lo + FCH], in1=t8ps[:], op=mult
                 )
             t8list.append(t9sb)

         # --- t7 direct: t7[(d,c1), c] accumulated as 3 col-groups ---
         t7ps = ps_t7.tile([C, 3 * C], F32, tag="t7")
         for d in range(3):
             for j in range(NCH):
                 nc.tensor.matmul(
                     t7ps[:, d * C : (d + 1) * C],
                     t2t[:, (3 * j + d) * C : (3 * j + d + 1) * C],
                     t6t[:, j * C : (j + 1) * C],
                     start=(j == 0),
                     stop=(j == NCH - 1),
                 )
         t7d_sb = p_t7sb.tile([C, 3 * C], BF, tag="t7d")
         nc.scalar.copy(t7d_sb[:], t7ps[:])

         # --- output stage ---
         # t10 is a standard accumulation group (d=1 starts, d=0/d=2 add);
         # t9 = x*t8 and the final add are split between DVE and ACT per f
         # to balance engine load.
         outsb = p_out.tile([C, HW], BF, tag="outp")
         for f in range(NF):
             lo = f * FCH
             t9sb = t8list[f]
             t10ps = ps_f.tile([C, FCH], F32, tag="fps")
             h0 = f * 8
             xp0 = xpr[:, h0 : h0 + 8, 0:W]          # x[h, w-2] w/ zeros
             xp2 = xpr[:, h0 : h0 + 8, 4 : 4 + W]    # x[h, w+2] w/ zeros
             nc.tensor.matmul(
                 t10ps[:], t7d_sb[:, C : 2 * C], xbf[:, lo : lo + FCH],
                 start=True, stop=False,
             )
             nc.tensor.matmul(
                 t10ps[:], t7d_sb[:, 0:C], xp0,
                 start=False, stop=False,
             )
             nc.tensor.matmul(
                 t10ps[:], t7d_sb[:, 2 * C : 3 * C], xp2,
                 start=False, stop=True,
             )
             t10sb = p_fsb.tile([C, FCH], BF, tag="t10sb")
             nc.scalar.copy(t10sb[:], t10ps[:])
             nc.vector.tensor_tensor(
                 out=outsb[:, lo : lo + FCH], in0=t9sb[:], in1=t10sb[:], op=addop
             )
         nc.sync.dma_start(out[s], outsb[:])

    if loop > 1:
        with tc.For_i(0, loop, 1):
            emit_samples()
    else:
        emit_samples()


def build(repeat=1, loop=1):
    if ("nc", repeat, loop) in _CACHE:
        return _CACHE[("nc", repeat, loop)]
    from contextlib import ExitStack

    import concourse.bass as bass
    import concourse.tile as tile
    from concourse import bacc, mybir

    BF = mybir.dt.bfloat16
    F32 = mybir.dt.float32
    nc = bacc.Bacc("TRN2", target_bir_lowering=False, debug=False)
    xs = nc.dram_tensor("xs", [NS, C, HW], BF, kind="ExternalInput").ap()
    p2wt = nc.dram_tensor("p2wt", [PCH, NCH * 3 * C], BF, kind="ExternalInput").ap()
    w2t = nc.dram_tensor("w2t", [C, C], BF, kind="ExternalInput").ap()
    ident = nc.dram_tensor("ident", [C, C], BF, kind="ExternalInput").ap()
    out = nc.dram_tensor("out", [NS, C, HW], BF, kind="ExternalOutput").ap()

    with tile.TileContext(nc) as tc:
        with __import__("contextlib").ExitStack() as ctx:
            _body(
                tc, bass, mybir, xs, p2wt, w2t, ident, out, BF, F32, ctx,
                repeat, loop,
            )
    nc.compile()
    _CACHE[("nc", repeat, loop)] = nc
    return nc


def host_inputs(x, p2w, p5w, conv_w):
    """Shard + prep per-core input maps from full inputs."""
    x = np.asarray(x, dtype=np.float32).reshape(N, C, HW).astype(BF16NP)
    x = np.ascontiguousarray(x)
    p2w_ = (np.asarray(p2w, dtype=np.float32)[0] * SCALE).reshape(C, 3, HW)
    a = p2w_.transpose(2, 1, 0)                      # (p, d, c1)
    a = a.reshape(NCH, PCH, 3, C)                    # (j, pl, d, c1)
    a = np.ascontiguousarray(a.transpose(1, 0, 2, 3)).reshape(PCH, NCH * 3 * C)
    p2wt = a.astype(BF16NP)
    p5 = np.asarray(p5w, dtype=np.float32).reshape(C)
    cw = np.asarray(conv_w, dtype=np.float32)        # (C//G, C)
    W2 = cw[np.arange(C) % (C // G)] * p5[None, :]   # (c, c')
    w2t = np.ascontiguousarray(W2.T).astype(BF16NP)  # (c', c)
    ident = np.eye(C, dtype=BF16NP)
    in_maps = [
        {
            "xs": np.ascontiguousarray(x[i * NS : (i + 1) * NS]),
            "p2wt": p2wt,
            "w2t": w2t,
            "ident": ident,
        }
        for i in range(NCORES)
    ]
    return in_maps


def _get_runner(repeat=1, loop=1):
    """Build (once) a persistent jitted shard_map executable over 8 cores."""
    if ("runner", repeat, loop) in _CACHE:
        return _CACHE[("runner", repeat, loop)]
    import jax
    from jax.sharding import Mesh, PartitionSpec
    from jax.experimental.shard_map import shard_map
    from concourse import bass2jax, mybir

    nc = build(repeat, loop)
    bass2jax.install_neuronx_cc_hook()

    partition_name = nc.partition_id_tensor.name if nc.partition_id_tensor else None
    in_names, out_names, out_avals, zero_outs = [], [], [], []
    for alloc in nc.m.functions[0].allocations:
        if not isinstance(alloc, mybir.MemoryLocationSet):
            continue
        name = alloc.memorylocations[0].name
        if alloc.kind == "ExternalInput":
            if name != partition_name:
                in_names.append(name)
        elif alloc.kind == "ExternalOutput":
            shape = tuple(alloc.tensor_shape)
            dtype = mybir.dt.np(alloc.dtype)
            out_avals.append(jax.core.ShapedArray(shape, dtype))
            zero_outs.append(np.zeros(shape, dtype))
            out_names.append(name)
    n_params = len(in_names)
    n_outs = len(out_avals)
    all_in_names = list(in_names) + list(out_names)
    if partition_name is not None:
        all_in_names.append(partition_name)
    donate = tuple(range(n_params, n_params + n_outs))

    def _body(*args):
        operands = list(args)
        if partition_name is not None:
            operands.append(bass2jax.partition_id_tensor())
        outs = bass2jax._bass_exec_p.bind(
            *operands,
            out_avals=tuple(out_avals),
            in_names=tuple(all_in_names),
            out_names=tuple(out_names),
            lowering_input_output_aliases=(),
            sim_require_finite=True,
            sim_require_nnan=True,
            nc=nc,
        )
        return tuple(outs)

    devices = jax.devices()[:NCORES]
    mesh = Mesh(np.asarray(devices), ("core",))
    in_specs = (PartitionSpec("core"),) * (n_params + n_outs)
    out_specs = (PartitionSpec("core"),) * n_outs
    sharded = jax.jit(
        shard_map(
            _body, mesh=mesh, in_specs=in_specs, out_specs=out_specs, check_rep=False
        ),
        donate_argnums=donate,
        keep_unused=True,
    )
    runner = {
        "fn": sharded,
        "in_names": in_names,
        "out_names": out_names,
        "out_avals": out_avals,
        "mesh": mesh,
        "n_params": n_params,
    }
    _CACHE[("runner", repeat, loop)] = runner
    return runner


def _concat_inputs(runner, in_maps):
    return [
        np.concatenate([np.asarray(m[name]) for m in in_maps], axis=0)
        for name in runner["in_names"]
    ]


def _zero_bufs(runner):
    return [
        np.zeros((NCORES * a.shape[0], *a.shape[1:]), a.dtype)
        for a in runner["out_avals"]
    ]


def run_fast(in_maps):
    """Execute via the cached jitted executable; returns list of per-core dicts."""
    runner = _get_runner()
    out_arrs = runner["fn"](*_concat_inputs(runner, in_maps), *_zero_bufs(runner))
    res = []
    for c in range(NCORES):
        res.append(
            {
                name: np.asarray(out_arrs[i]).reshape(
                    NCORES, *runner["out_avals"][i].shape
                )[c]
                for i, name in enumerate(runner["out_names"])
            }
        )
    return res


def run(in_maps, trace=False, **kw):
    from concourse.bass_utils import run_bass_kernel_spmd

    nc = build()
    return run_bass_kernel_spmd(nc, in_maps, list(range(NCORES)), trace=trace, **kw)


def bench(in_maps, iters=30):
    """Pipelined timing of the device executable. Returns sec/iter."""
    import time

    import jax
    from jax.sharding import NamedSharding, PartitionSpec

    runner = _get_runner()
    sh = NamedSharding(runner["mesh"], PartitionSpec("core"))
    dev_in = [jax.device_put(a, sh) for a in _concat_inputs(runner, in_maps)]
    # donated output buffers are consumed per call: pre-stage iters copies
    zsets = [
        [jax.device_put(z, sh) for z in _zero_bufs(runner)] for _ in range(iters + 2)
    ]
    for z in zsets[0]:
        z.block_until_ready()
    # warmup
    out = runner["fn"](*dev_in, *zsets[-1])
    jax.block_until_ready(out)
    out = runner["fn"](*dev_in, *zsets[-2])
    jax.block_until_ready(out)
    t0 = time.perf_counter()
    outs = []
    for k in range(iters):
        outs.append(runner["fn"](*dev_in, *zsets[k]))
    jax.block_until_ready(outs)
    dt = (time.perf_counter() - t0) / iters
    return dt


def bench_repeat(in_maps, R=13, reps=12, iters=4, inner=4):
    """Per-kernel time from a hardware-looped program: loop=R vs loop=1,
    each with `inner` kernels unrolled in the loop body. The two programs
    are byte-identical except the loop trip count, so per-call dispatch
    overhead is identical and cancels in the difference; measurement
    bursts are interleaved so slow drift cancels too. Outputs are chained
    back in as donated buffers so no host->device upload is timed."""
    import time

    import jax
    from jax.sharding import NamedSharding, PartitionSpec

    state = {}
    for rep in (1, R):
        runner = _get_runner(inner, rep)
        sh = NamedSharding(runner["mesh"], PartitionSpec("core"))
        dev_in = [jax.device_put(a, sh) for a in _concat_inputs(runner, in_maps)]
        outs = [jax.device_put(z, sh) for z in _zero_bufs(runner)]
        jax.block_until_ready(dev_in)
        jax.block_until_ready(outs)
        outs = runner["fn"](*dev_in, *outs)  # warmup + first chain
        jax.block_until_ready(outs)
        state[rep] = [runner, dev_in, outs, []]

    for _ in range(reps):
        for rep in (1, R):
            runner, dev_in, outs, ts = state[rep]
            t0 = time.perf_counter()
            for _ in range(iters):
                outs = runner["fn"](*dev_in, *outs)
            jax.block_until_ready(outs)
            ts.append((time.perf_counter() - t0) / iters)
            state[rep][2] = outs

    t1s = sorted(state[1][3])
    tRs = sorted(state[R][3])
    k = max(3, len(t1s) // 2)
    t1 = sum(t1s[:k]) / k
    tR = sum(tRs[:k]) / k
    per = (tR - t1) / ((R - 1) * inner)
    return per, t1, tR


def kernel(x, p2w, p5w, conv_w):
    in_maps = host_inputs(x, p2w, p5w, conv_w)
    res = run_fast(in_maps)
    outs = [np.asarray(res[i]["out"]) for i in range(NCORES)]
    return (
        np.concatenate(outs, axis=0).reshape(N, C, H, W).astype(np.float32)
    )


# revision 29
# speedup vs baseline: 1.0855x; 1.0855x over previous
"""Trainium2 Bass kernel for the sparse_attention nn problem.

Math (per sample n, all shapes per sample):
  t1_d = x shifted by 2*(d-1) rows (zero pad), d in {0,1,2}
  t2_d = p2w_d * t1_d
  t3_d = x shifted by 2*(d-1) cols (zero pad)
  t4   = roll(x, 1, rows);  t6 = t4 + x
  t7[k=(d,c1), c] = sum_p t2_d[c1,p] * t6[c,p] / 56
  t8full = W'' @ t4 with W''[c,c'] = conv_w[c%4,c'] * p5w[c']   (t9 = t8full*x)
  t10[c,p] = sum_{d,c1} t7[(d,c1),c] * t3_d[c1,p] / sqrt(384)
  out = t9 + t10
k-order is permuted (d-major) consistently in t7/t10 -> result unchanged.
Scales folded into p2w on host: p2w' = p2w / (56*sqrt(384)).

Sharding: pure data parallel over batch (4 samples per core, 8 cores).

v2 design notes:
 - x is cast to bf16 on the host; output written bf16 (halves DMA traffic).
 - x^T built on PE (28 transposes of (128x112) into 4 one-bank PSUM groups),
   evacuated to SBUF by ACT.
 - t2^T = p2w^T * x^T(d-shifted): 3 DVE ops (free-dim chunk shifts).
 - t6 = x + roll(x) built on DVE in natural layout (free-dim shifts),
   transposed on PE like x.
 - t7 accumulated directly in the [(d,c1), c] orientation (3 sequential
   PSUM col-groups, 84 narrow matmuls) -> single ACT evacuation, no
   extra transposes.
 - t3 col shifts are folded into the t10 matmuls by reading a zero-halo
   padded copy of x (xpad, row stride 60) -- zero materialization of t3.
 - output stage per 448-col chunk: t8 matmul; t9 = x*t8 on DVE (from PSUM
   or via an ACT-staged bf16 copy, alternating to balance engines); t10 as
   a standard 3-matmul accumulation group; ACT evacuates t10 as bf16 and
   DVE adds t9 + t10 into the bf16 output tile.
"""

import math
import numpy as np
import ml_dtypes

N, C, H, W, G = 32, 128, 56, 56, 32
HW = H * W                # 3136
NCORES = 8
NS = N // NCORES          # 4 samples per core
PCH = 2 * W               # 112 = p-chunk (2 image rows)
NCH = HW // PCH           # 28 chunks
HP = 56                   # half-chunk (1 image row) partition offset
NG = 4                    # psum transpose groups
GB = NCH // NG            # 7 blocks per group (1 bank)
FCH = 448                 # t10/t8 free chunk (8 rows)
NF = HW // FCH            # 7
SCALE = 1.0 / (56.0 * math.sqrt(384.0))
BF16NP = ml_dtypes.bfloat16

_CACHE = {}


def _body(tc, bass, mybir, xs, p2wt, w2t, ident, out, BF, F32, ctx, repeat=1, loop=1):
    nc = tc.nc
    mult = mybir.AluOpType.mult
    addop = mybir.AluOpType.add

    consts = ctx.enter_context(tc.tile_pool(name="consts", bufs=1))
    p_xbf = ctx.enter_context(tc.tile_pool(name="xbf", bufs=3))
    p_xtsb = ctx.enter_context(tc.tile_pool(name="xtsb", bufs=3))
    p_t6n = ctx.enter_context(tc.tile_pool(name="t6nn", bufs=2))
    p_t6t = ctx.enter_context(tc.tile_pool(name="t6t", bufs=3))
    p_t7sb = ctx.enter_context(tc.tile_pool(name="t7sb", bufs=3))
    p_out = ctx.enter_context(tc.tile_pool(name="outp", bufs=3))
    p_fsb = ctx.enter_context(tc.tile_pool(name="fsb", bufs=3))
    ps_tr = ctx.enter_context(tc.tile_pool(name="pstr", bufs=3, space="PSUM"))
    ps_t7 = ctx.enter_context(tc.tile_pool(name="pst7", bufs=2, space="PSUM"))
    ps_f = ctx.enter_context(tc.tile_pool(name="psf", bufs=3, space="PSUM"))

    p2wt_t = consts.tile([PCH, NCH * 3 * C], BF)
    nc.sync.dma_start(p2wt_t[:], p2wt)
    w2t_t = consts.tile([C, C], BF)
    nc.sync.dma_start(w2t_t[:], w2t)
    id_t = consts.tile([C, C], BF)
    nc.sync.dma_start(id_t[:], ident)

    # two persistent t2t buffers (alternate across samples); their boundary
    # blocks (j=0,d=0) and (j=27,d=2) are zero forever (shifted-x zero pad).
    t2ts = []
    for k in range(2):
        t2t = consts.tile([PCH, NCH * 3 * C], BF, tag=f"t2t{k}")
        nc.gpsimd.memset(t2t[:, 0:C], 0.0)
        nc.gpsimd.memset(t2t[:, (NCH * 3 - 1) * C : NCH * 3 * C], 0.0)
        t2ts.append(t2t)
    pwr = p2wt_t[:].rearrange("p (j x) -> p j x", x=3 * C)

    # x with a 2-col zero halo per row (row stride 60) for the t3 col shifts
    WP = W + 4
    xpads = []
    for k in range(2):
        xp = consts.tile([C, H * WP], BF, tag=f"xpad{k}")
        xpr = xp[:].rearrange("c (h w) -> c h w", w=WP)
        nc.gpsimd.memset(xpr[:, :, 0:2], 0.0)
        nc.gpsimd.memset(xpr[:, :, W + 2 : WP], 0.0)
        xpads.append(xp)

    def emit_samples():
     for si, s in enumerate([s for _ in range(repeat) for s in range(NS)]):
         xbf = p_xbf.tile([C, HW], BF, tag="xbf")
         nc.sync.dma_start(xbf[:], xs[s])

         # padded copy for the t10 col-shifted moving operands
         xpad = xpads[si % 2]
         xpr = xpad[:].rearrange("c (h w) -> c h w", w=WP)
         nc.gpsimd.tensor_copy(
             xpr[:, :, 2 : W + 2],
             xbf[:].rearrange("c (h w) -> c h w", w=W),
         )

         # --- transpose x into PSUM: 4 groups of 7 chunks (1 bank each) ---
         xt_sb = p_xtsb.tile([PCH, NCH * C], BF, tag="xtsb")
         for g in range(NG):
             xtg = ps_tr.tile([PCH, GB * C], BF, tag="tr")
             for r in range(GB):
                 j = g * GB + r
                 nc.tensor.transpose(
                     xtg[:, r * C : (r + 1) * C],
                     xbf[:, j * PCH : (j + 1) * PCH],
                     id_t[:],
                 )
             nc.scalar.copy(
                 xt_sb[:, g * GB * C : (g + 1) * GB * C], xtg[:, 0 : GB * C]
             )

         # --- t2^T = p2w^T * x^T(shifted) --- 3 DVE ops via chunk shifts
         t2t = t2ts[si % 2]
         t2r = t2t[:].rearrange("p (j x) -> p j x", x=3 * C)
         for d in range(3):
             j0 = 1 if d == 0 else 0
             j1 = NCH - 1 if d == 2 else NCH
             nc.vector.tensor_tensor(
                 out=t2r[:, j0:j1, d * C : (d + 1) * C],
                 in0=pwr[:, j0:j1, d * C : (d + 1) * C],
                 in1=xt_sb[:, (j0 + d - 1) * C : (j1 + d - 1) * C].rearrange(
                     "p (j c) -> p j c", c=C
                 ),
                 op=mult,
             )

         # --- t6 = x + roll(x, 1 row) in natural layout, then transpose ---
         t6n = p_t6n.tile([C, HW], BF, tag="t6n")
         nc.vector.tensor_tensor(
             out=t6n[:, W:HW], in0=xbf[:, W:HW], in1=xbf[:, 0 : HW - W], op=addop
         )
         nc.vector.tensor_tensor(
             out=t6n[:, 0:W], in0=xbf[:, 0:W], in1=xbf[:, HW - W : HW], op=addop
         )
         t6t = p_t6t.tile([PCH, NCH * C], BF, tag="t6t")
         for g in range(NG):
             t6g = ps_tr.tile([PCH, GB * C], BF, tag="tr")
             for r in range(GB):
                 j = g * GB + r
                 nc.tensor.transpose(
                     t6g[:, r * C : (r + 1) * C],
                     t6n[:, j * PCH : (j + 1) * PCH],
                     id_t[:],
                 )
             nc.scalar.copy(
                 t6t[:, g * GB * C : (g + 1) * GB * C], t6g[:, 0 : GB * C]
             )

         # --- t7 direct: t7[(d,c1), c] accumulated as 3 col-groups ---
         t7ps = ps_t7.tile([C, 3 * C], F32, tag="t7")
         for d in range(3):
             for j in range(NCH):
                 nc.tensor.matmul(
                     t7ps[:, d * C : (d + 1) * C],
                     t2t[:, (3 * j + d) * C : (3 * j + d + 1) * C],
                     t6t[:, j * C : (j + 1) * C],
                     start=(j == 0),
                     stop=(j == NCH - 1),
                 )
         t7d_sb = p_t7sb.tile([C, 3 * C], BF, tag="t7d")
         nc.scalar.copy(t7d_sb[:], t7ps[:])

         # --- output stage ---
         # t10 is a standard accumulation group (d=1 starts, d=0/d=2 add);
         # t9 = x*t8 and the final add are split between DVE and ACT per f
         # to balance engine load.
         outsb = p_out.tile([C, HW], BF, tag="outp")
         for f in range(NF):
             lo = f * FCH
             t8ps = ps_f.tile([C, FCH], F32, tag="fps")
             if f == 0:
                 nc.tensor.matmul(
                     t8ps[:, 0:W], w2t_t[:], xbf[:, HW - W : HW],
                     start=True, stop=False, skip_group_check=True,
                 )
                 nc.tensor.matmul(
                     t8ps[:, W:FCH], w2t_t[:], xbf[:, 0 : FCH - W],
                     start=True, stop=True, skip_group_check=True,
                 )
             else:
                 nc.tensor.matmul(
                     t8ps[:], w2t_t[:], xbf[:, lo - W : lo + FCH - W],
                     start=True, stop=True,
                 )
             t9sb = p_fsb.tile([C, FCH], BF, tag="t9sb")
             use_act_t8 = f % 2 == 0
             if use_act_t8:
                 t8sb = p_fsb.tile([C, FCH], BF, tag="t8sb")
                 nc.scalar.copy(t8sb[:], t8ps[:])
                 nc.vector.tensor_tensor(
                     out=t9sb[:], in0=xbf[:, lo : lo + FCH], in1=t8sb[:], op=mult
                 )
             else:
                 nc.vector.tensor_tensor(
                     out=t9sb[:], in0=xbf[:, lo : lo + FCH], in1=t8ps[:], op=mult
                 )
             t10ps = ps_f.tile([C, FCH], F32, tag="fps")
             h0 = f * 8
             xp0 = xpr[:, h0 : h0 + 8, 0:W]          # x[h, w-2] w/ zeros
             xp2 = xpr[:, h0 : h0 + 8, 4 : 4 + W]    # x[h, w+2] w/ zeros
             nc.tensor.matmul(
                 t10ps[:], t7d_sb[:, C : 2 * C], xbf[:, lo : lo + FCH],
                 start=True, stop=False,
             )
             nc.tensor.matmul(
                 t10ps[:], t7d_sb[:, 0:C], xp0,
                 start=False, stop=False,
             )
             nc.tensor.matmul(
                 t10ps[:], t7d_sb[:, 2 * C : 3 * C], xp2,
                 start=False, stop=True,
             )
             t10sb = p_fsb.tile([C, FCH], BF, tag="t10sb")
             nc.scalar.copy(t10sb[:], t10ps[:])
             nc.vector.tensor_tensor(
                 out=outsb[:, lo : lo + FCH], in0=t9sb[:], in1=t10sb[:], op=addop
             )
         nc.sync.dma_start(out[s], outsb[:])

    if loop > 1:
        with tc.For_i(0, loop, 1):
            emit_samples()
    else:
        emit_samples()


def build(repeat=1, loop=1):
    if ("nc", repeat, loop) in _CACHE:
        return _CACHE[("nc", repeat, loop)]
    from contextlib import ExitStack

    import concourse.bass as bass
    import concourse.tile as tile
    from concourse import bacc, mybir

    BF = mybir.dt.bfloat16
    F32 = mybir.dt.float32
    nc = bacc.Bacc("TRN2", target_bir_lowering=False, debug=False)
    xs = nc.dram_tensor("xs", [NS, C, HW], BF, kind="ExternalInput").ap()
    p2wt = nc.dram_tensor("p2wt", [PCH, NCH * 3 * C], BF, kind="ExternalInput").ap()
    w2t = nc.dram_tensor("w2t", [C, C], BF, kind="ExternalInput").ap()
    ident = nc.dram_tensor("ident", [C, C], BF, kind="ExternalInput").ap()
    out = nc.dram_tensor("out", [NS, C, HW], BF, kind="ExternalOutput").ap()

    with tile.TileContext(nc) as tc:
        with __import__("contextlib").ExitStack() as ctx:
            _body(
                tc, bass, mybir, xs, p2wt, w2t, ident, out, BF, F32, ctx,
                repeat, loop,
            )
    nc.compile()
    _CACHE[("nc", repeat, loop)] = nc
    return nc


def host_inputs(x, p2w, p5w, conv_w):
    """Shard + prep per-core input maps from full inputs."""
    x = np.asarray(x, dtype=np.float32).reshape(N, C, HW).astype(BF16NP)
    x = np.ascontiguousarray(x)
    p2w_ = (np.asarray(p2w, dtype=np.float32)[0] * SCALE).reshape(C, 3, HW)
    a = p2w_.transpose(2, 1, 0)                      # (p, d, c1)
    a = a.reshape(NCH, PCH, 3, C)                    # (j, pl, d, c1)
    a = np.ascontiguousarray(a.transpose(1, 0, 2, 3)).reshape(PCH, NCH * 3 * C)
    p2wt = a.astype(BF16NP)
    p5 = np.asarray(p5w, dtype=np.float32).reshape(C)
    cw = np.asarray(conv_w, dtype=np.float32)        # (C//G, C)
    W2 = cw[np.arange(C) % (C // G)] * p5[None, :]   # (c, c')
    w2t = np.ascontiguousarray(W2.T).astype(BF16NP)  # (c', c)
    ident = np.eye(C, dtype=BF16NP)
    in_maps = [
        {
            "xs": np.ascontiguousarray(x[i * NS : (i + 1) * NS]),
            "p2wt": p2wt,
            "w2t": w2t,
            "ident": ident,
        }
        for i in range(NCORES)
    ]
    return in_maps


def _get_runner(repeat=1, loop=1):
    """Build (once) a persistent jitted shard_map executable over 8 cores."""
    if ("runner", repeat, loop) in _CACHE:
        return _CACHE[("runner", repeat, loop)]
    import jax
    from jax.sharding import Mesh, PartitionSpec
    from jax.experimental.shard_map import shard_map
    from concourse import bass2jax, mybir

    nc = build(repeat, loop)
    bass2jax.install_neuronx_cc_hook()

    partition_name = nc.partition_id_tensor.name if nc.partition_id_tensor else None
    in_names, out_names, out_avals, zero_outs = [], [], [], []
    for alloc in nc.m.functions[0].allocations:
        if not isinstance(alloc, mybir.MemoryLocationSet):
            continue
        name = alloc.memorylocations[0].name
        if alloc.kind == "ExternalInput":
            if name != partition_name:
                in_names.append(name)
        elif alloc.kind == "ExternalOutput":
            shape = tuple(alloc.tensor_shape)
            dtype = mybir.dt.np(alloc.dtype)
            out_avals.append(jax.core.ShapedArray(shape, dtype))
            zero_outs.append(np.zeros(shape, dtype))
            out_names.append(name)
    n_params = len(in_names)
    n_outs = len(out_avals)
    all_in_names = list(in_names) + list(out_names)
    if partition_name is not None:
        all_in_names.append(partition_name)
    donate = tuple(range(n_params, n_params + n_outs))

    def _body(*args):
        operands = list(args)
        if partition_name is not None:
            operands.append(bass2jax.partition_id_tensor())
        outs = bass2jax._bass_exec_p.bind(
            *operands,
            out_avals=tuple(out_avals),
            in_names=tuple(all_in_names),
            out_names=tuple(out_names),
            lowering_input_output_aliases=(),
            sim_require_finite=True,
            sim_require_nnan=True,
            nc=nc,
        )
        return tuple(outs)

    devices = jax.devices()[:NCORES]
    mesh = Mesh(np.asarray(devices), ("core",))
    in_specs = (PartitionSpec("core"),) * (n_params + n_outs)
    out_specs = (PartitionSpec("core"),) * n_outs
    sharded = jax.jit(
        shard_map(
            _body, mesh=mesh, in_specs=in_specs, out_specs=out_specs, check_rep=False
        ),
        donate_argnums=donate,
        keep_unused=True,
    )
    runner = {
        "fn": sharded,
        "in_names": in_names,
        "out_names": out_names,
        "out_avals": out_avals,
        "mesh": mesh,
        "n_params": n_params,
    }
    _CACHE[("runner", repeat, loop)] = runner
    return runner


def _concat_inputs(runner, in_maps):
    return [
        np.concatenate([np.asarray(m[name]) for m in in_maps], axis=0)
        for name in runner["in_names"]
    ]


def _zero_bufs(runner):
    return [
        np.zeros((NCORES * a.shape[0], *a.shape[1:]), a.dtype)
        for a in runner["out_avals"]
    ]


def run_fast(in_maps):
    """Execute via the cached jitted executable; returns list of per-core dicts."""
    runner = _get_runner()
    out_arrs = runner["fn"](*_concat_inputs(runner, in_maps), *_zero_bufs(runner))
    res = []
    for c in range(NCORES):
        res.append(
            {
                name: np.asarray(out_arrs[i]).reshape(
                    NCORES, *runner["out_avals"][i].shape
                )[c]
                for i, name in enumerate(runner["out_names"])
            }
        )
    return res


def run(in_maps, trace=False, **kw):
    from concourse.bass_utils import run_bass_kernel_spmd

    nc = build()
    return run_bass_kernel_spmd(nc, in_maps, list(range(NCORES)), trace=trace, **kw)


def bench(in_maps, iters=30):
    """Pipelined timing of the device executable. Returns sec/iter."""
    import time

    import jax
    from jax.sharding import NamedSharding, PartitionSpec

    runner = _get_runner()
    sh = NamedSharding(runner["mesh"], PartitionSpec("core"))
    dev_in = [jax.device_put(a, sh) for a in _concat_inputs(runner, in_maps)]
    # donated output buffers are consumed per call: pre-stage iters copies
    zsets = [
        [jax.device_put(z, sh) for z in _zero_bufs(runner)] for _ in range(iters + 2)
    ]
    for z in zsets[0]:
        z.block_until_ready()
    # warmup
    out = runner["fn"](*dev_in, *zsets[-1])
    jax.block_until_ready(out)
    out = runner["fn"](*dev_in, *zsets[-2])
    jax.block_until_ready(out)
    t0 = time.perf_counter()
    outs = []
    for k in range(iters):
        outs.append(runner["fn"](*dev_in, *zsets[k]))
    jax.block_until_ready(outs)
    dt = (time.perf_counter() - t0) / iters
    return dt


def bench_repeat(in_maps, R=13, reps=12, iters=4, inner=4):
    """Per-kernel time from a hardware-looped program: loop=R vs loop=1,
    each with `inner` kernels unrolled in the loop body. The two programs
    are byte-identical except the loop trip count, so per-call dispatch
    overhead is identical and cancels in the difference; measurement
    bursts are interleaved so slow drift cancels too. Outputs are chained
    back in as donated buffers so no host->device upload is timed."""
    import time

    import jax
    from jax.sharding import NamedSharding, PartitionSpec

    state = {}
    for rep in (1, R):
        runner = _get_runner(inner, rep)
        sh = NamedSharding(runner["mesh"], PartitionSpec("core"))
        dev_in = [jax.device_put(a, sh) for a in _concat_inputs(runner, in_maps)]
        outs = [jax.device_put(z, sh) for z in _zero_bufs(runner)]
        jax.block_until_ready(dev_in)
        jax.block_until_ready(outs)
        outs = runner["fn"](*dev_in, *outs)  # warmup + first chain
        jax.block_until_ready(outs)
        state[rep] = [runner, dev_in, outs, []]

    for _ in range(reps):
        for rep in (1, R):
            runner, dev_in, outs, ts = state[rep]
            t0 = time.perf_counter()
            for _ in range(iters):
                outs = runner["fn"](*dev_in, *outs)
            jax.block_until_ready(outs)
            ts.append((time.perf_counter() - t0) / iters)
            state[rep][2] = outs

    t1s = sorted(state[1][3])
    tRs = sorted(state[R][3])
    k = max(3, len(t1s) // 2)
    t1 = sum(t1s[:k]) / k
    tR = sum(tRs[:k]) / k
    per = (tR - t1) / ((R - 1) * inner)
    return per, t1, tR


def kernel(x, p2w, p5w, conv_w):
    in_maps = host_inputs(x, p2w, p5w, conv_w)
    res = run_fast(in_maps)
    outs = [np.asarray(res[i]["out"]) for i in range(NCORES)]
    return (
        np.concatenate(outs, axis=0).reshape(N, C, H, W).astype(np.float32)
    )


# revision 32
# speedup vs baseline: 1.1048x; 1.0178x over previous
"""Trainium2 Bass kernel for the sparse_attention nn problem.

Math (per sample n, all shapes per sample):
  t1_d = x shifted by 2*(d-1) rows (zero pad), d in {0,1,2}
  t2_d = p2w_d * t1_d
  t3_d = x shifted by 2*(d-1) cols (zero pad)
  t4   = roll(x, 1, rows);  t6 = t4 + x
  t7[k=(d,c1), c] = sum_p t2_d[c1,p] * t6[c,p] / 56
  t8full = W'' @ t4 with W''[c,c'] = conv_w[c%4,c'] * p5w[c']   (t9 = t8full*x)
  t10[c,p] = sum_{d,c1} t7[(d,c1),c] * t3_d[c1,p] / sqrt(384)
  out = t9 + t10
k-order is permuted (d-major) consistently in t7/t10 -> result unchanged.
Scales folded into p2w on host: p2w' = p2w / (56*sqrt(384)).

Sharding: pure data parallel over batch (4 samples per core, 8 cores).

v2 design notes:
 - x is cast to bf16 on the host; output written bf16 (halves DMA traffic).
 - x^T built on PE (28 transposes of (128x112) into 4 one-bank PSUM groups),
   evacuated to SBUF by ACT.
 - t2^T = p2w^T * x^T(d-shifted): 3 DVE ops (free-dim chunk shifts).
 - t6 = x + roll(x) built on DVE in natural layout (free-dim shifts),
   transposed on PE like x.
 - t7 accumulated directly in the [(d,c1), c] orientation (3 sequential
   PSUM col-groups, 84 narrow matmuls) -> single ACT evacuation, no
   extra transposes.
 - t3 col shifts are folded into the t10 matmuls by reading a zero-halo
   padded copy of x (xpad, row stride 60) -- zero materialization of t3.
 - output stage per 448-col chunk: t8 matmul; t9 = x*t8 on DVE (from PSUM
   or via an ACT-staged bf16 copy, alternating to balance engines); t10 as
   a standard 3-matmul accumulation group; ACT evacuates t10 as bf16 and
   DVE adds t9 + t10 into the bf16 output tile.
"""

import math
import numpy as np
import ml_dtypes

N, C, H, W, G = 32, 128, 56, 56, 32
HW = H * W                # 3136
NCORES = 8
NS = N // NCORES          # 4 samples per core
PCH = 2 * W               # 112 = p-chunk (2 image rows)
NCH = HW // PCH           # 28 chunks
HP = 56                   # half-chunk (1 image row) partition offset
NG = 4                    # psum transpose groups
GB = NCH // NG            # 7 blocks per group (1 bank)
FCH = 448                 # t10/t8 free chunk (8 rows)
NF = HW // FCH            # 7
SCALE = 1.0 / (56.0 * math.sqrt(384.0))
BF16NP = ml_dtypes.bfloat16

_CACHE = {}


def _body(tc, bass, mybir, xs, p2wt, w2t, ident, out, BF, F32, ctx, repeat=1, loop=1):
    nc = tc.nc
    mult = mybir.AluOpType.mult
    addop = mybir.AluOpType.add

    consts = ctx.enter_context(tc.tile_pool(name="consts", bufs=1))
    p_xbf = ctx.enter_context(tc.tile_pool(name="xbf", bufs=3))
    p_xtsb = ctx.enter_context(tc.tile_pool(name="xtsb", bufs=3))
    p_t6n = ctx.enter_context(tc.tile_pool(name="t6nn", bufs=2))
    p_t6t = ctx.enter_context(tc.tile_pool(name="t6t", bufs=3))
    p_t7sb = ctx.enter_context(tc.tile_pool(name="t7sb", bufs=3))
    p_out = ctx.enter_context(tc.tile_pool(name="outp", bufs=3))
    p_fsb = ctx.enter_context(tc.tile_pool(name="fsb", bufs=3))
    ps_tr = ctx.enter_context(tc.tile_pool(name="pstr", bufs=3, space="PSUM"))
    ps_t7 = ctx.enter_context(tc.tile_pool(name="pst7", bufs=2, space="PSUM"))
    ps_f = ctx.enter_context(tc.tile_pool(name="psf", bufs=3, space="PSUM"))

    p2wt_t = consts.tile([PCH, NCH * 3 * C], BF)
    nc.sync.dma_start(p2wt_t[:], p2wt)
    w2t_t = consts.tile([C, C], BF)
    nc.sync.dma_start(w2t_t[:], w2t)
    id_t = consts.tile([C, C], BF)
    nc.sync.dma_start(id_t[:], ident)

    # two persistent t2t buffers (alternate across samples); their boundary
    # blocks (j=0,d=0) and (j=27,d=2) are zero forever (shifted-x zero pad).
    t2ts = []
    for k in range(2):
        t2t = consts.tile([PCH, NCH * 3 * C], BF, tag=f"t2t{k}")
        nc.gpsimd.memset(t2t[:, 0:C], 0.0)
        nc.gpsimd.memset(t2t[:, (NCH * 3 - 1) * C : NCH * 3 * C], 0.0)
        t2ts.append(t2t)
    pwr = p2wt_t[:].rearrange("p (j x) -> p j x", x=3 * C)

    # x with a 2-col zero halo per row (row stride 60) for the t3 col shifts
    WP = W + 4
    xpads = []
    for k in range(2):
        xp = consts.tile([C, H * WP], BF, tag=f"xpad{k}")
        xpr = xp[:].rearrange("c (h w) -> c h w", w=WP)
        nc.gpsimd.memset(xpr[:, :, 0:2], 0.0)
        nc.gpsimd.memset(xpr[:, :, W + 2 : WP], 0.0)
        xpads.append(xp)

    def emit_samples():
     for si, s in enumerate([s for _ in range(repeat) for s in range(NS)]):
         xbf = p_xbf.tile([C, HW], BF, tag="xbf")
         nc.sync.dma_start(xbf[:], xs[s])

         # padded copy for the t10 col-shifted moving operands
         xpad = xpads[si % 2]
         xpr = xpad[:].rearrange("c (h w) -> c h w", w=WP)
         nc.gpsimd.tensor_copy(
             xpr[:, :, 2 : W + 2],
             xbf[:].rearrange("c (h w) -> c h w", w=W),
         )

         # --- transpose x into PSUM: 4 groups of 7 chunks (1 bank each) ---
         xt_sb = p_xtsb.tile([PCH, NCH * C], BF, tag="xtsb")
         for g in range(NG):
             xtg = ps_tr.tile([PCH, GB * C], BF, tag="tr")
             for r in range(GB):
                 j = g * GB + r
                 nc.tensor.transpose(
                     xtg[:, r * C : (r + 1) * C],
                     xbf[:, j * PCH : (j + 1) * PCH],
                     id_t[:],
                 )
             nc.scalar.copy(
                 xt_sb[:, g * GB * C : (g + 1) * GB * C], xtg[:, 0 : GB * C]
             )

         # --- t2^T = p2w^T * x^T(shifted) --- 3 DVE ops via chunk shifts
         t2t = t2ts[si % 2]
         t2r = t2t[:].rearrange("p (j x) -> p j x", x=3 * C)
         for d in range(3):
             j0 = 1 if d == 0 else 0
             j1 = NCH - 1 if d == 2 else NCH
             nc.vector.tensor_tensor(
                 out=t2r[:, j0:j1, d * C : (d + 1) * C],
                 in0=pwr[:, j0:j1, d * C : (d + 1) * C],
                 in1=xt_sb[:, (j0 + d - 1) * C : (j1 + d - 1) * C].rearrange(
                     "p (j c) -> p j c", c=C
                 ),
                 op=mult,
             )

         # --- t6 = x + roll(x, 1 row) in natural layout, then transpose ---
         t6n = p_t6n.tile([C, HW], BF, tag="t6n")
         nc.vector.tensor_tensor(
             out=t6n[:, W:HW], in0=xbf[:, W:HW], in1=xbf[:, 0 : HW - W], op=addop
         )
         nc.vector.tensor_tensor(
             out=t6n[:, 0:W], in0=xbf[:, 0:W], in1=xbf[:, HW - W : HW], op=addop
         )
         t6t = p_t6t.tile([PCH, NCH * C], BF, tag="t6t")
         for g in range(NG):
             t6g = ps_tr.tile([PCH, GB * C], BF, tag="tr")
             for r in range(GB):
                 j = g * GB + r
                 nc.tensor.transpose(
                     t6g[:, r * C : (r + 1) * C],
                     t6n[:, j * PCH : (j + 1) * PCH],
                     id_t[:],
                 )
             nc.scalar.copy(
                 t6t[:, g * GB * C : (g + 1) * GB * C], t6g[:, 0 : GB * C]
             )

         # --- t7 direct: t7[(d,c1), c] accumulated as 3 col-groups ---
         t7ps = ps_t7.tile([C, 3 * C], F32, tag="t7")
         for d in range(3):
             for j in range(NCH):
                 nc.tensor.matmul(
                     t7ps[:, d * C : (d + 1) * C],
                     t2t[:, (3 * j + d) * C : (3 * j + d + 1) * C],
                     t6t[:, j * C : (j + 1) * C],
                     start=(j == 0),
                     stop=(j == NCH - 1),
                 )
         t7d_sb = p_t7sb.tile([C, 3 * C], BF, tag="t7d")
         nc.scalar.copy(t7d_sb[:], t7ps[:])

         # --- output stage ---
         # t10 is a standard accumulation group (d=1 starts, d=0/d=2 add);
         # t9 = x*t8 and the final add are split between DVE and ACT per f
         # to balance engine load.
         outsb = p_out.tile([C, HW], BF, tag="outp")
         for f in range(NF):
             lo = f * FCH
             t8ps = ps_f.tile([C, FCH], F32, tag="fps")
             if f == 0:
                 nc.tensor.matmul(
                     t8ps[:, 0:W], w2t_t[:], xbf[:, HW - W : HW],
                     start=True, stop=False, skip_group_check=True,
                 )
                 nc.tensor.matmul(
                     t8ps[:, W:FCH], w2t_t[:], xbf[:, 0 : FCH - W],
                     start=True, stop=True, skip_group_check=True,
                 )
             else:
                 nc.tensor.matmul(
                     t8ps[:], w2t_t[:], xbf[:, lo - W : lo + FCH - W],
                     start=True, stop=True,
                 )
             t9sb = p_fsb.tile([C, FCH], BF, tag="t9sb")
             use_act_t8 = f % 2 == 0
             if use_act_t8:
                 t8sb = p_fsb.tile([C, FCH], BF, tag="t8sb")
                 nc.scalar.copy(t8sb[:], t8ps[:])
                 nc.vector.tensor_tensor(
                     out=t9sb[:], in0=xbf[:, lo : lo + FCH], in1=t8sb[:], op=mult
                 )
             else:
                 nc.vector.tensor_tensor(
                     out=t9sb[:], in0=xbf[:, lo : lo + FCH], in1=t8ps[:], op=mult
                 )
             t10ps = ps_f.tile([C, FCH], F32, tag="fps")
             h0 = f * 8
             xp0 = xpr[:, h0 : h0 + 8, 0:W]          # x[h, w-2] w/ zeros
             xp2 = xpr[:, h0 : h0 + 8, 4 : 4 + W]    # x[h, w+2] w/ zeros
             nc.tensor.matmul(
                 t10ps[:], t7d_sb[:, C : 2 * C], xbf[:, lo : lo + FCH],
                 start=True, stop=False,
             )
             nc.tensor.matmul(
                 t10ps[:], t7d_sb[:, 0:C], xp0,
                 start=False, stop=False,
             )
             nc.tensor.matmul(
                 t10ps[:], t7d_sb[:, 2 * C : 3 * C], xp2,
                 start=False, stop=True,
             )
             t10sb = p_fsb.tile([C, FCH], BF, tag="t10sb")
             nc.scalar.copy(t10sb[:], t10ps[:])
             nc.vector.tensor_tensor(
                 out=outsb[:, lo : lo + FCH], in0=t9sb[:], in1=t10sb[:], op=addop
             )
         nc.sync.dma_start(out[s], outsb[:])

    if loop > 1:
        with tc.For_i(0, loop, 1):
            emit_samples()
    else:
        emit_samples()


def build(repeat=1, loop=1):
    if ("nc", repeat, loop) in _CACHE:
        return _CACHE[("nc", repeat, loop)]
    from contextlib import ExitStack

    import concourse.bass as bass
    import concourse.tile as tile
    from concourse import bacc, mybir

    BF = mybir.dt.bfloat16
    F32 = mybir.dt.float32
    nc = bacc.Bacc("TRN2", target_bir_lowering=False, debug=False)
    xs = nc.dram_tensor("xs", [NS, C, HW], BF, kind="ExternalInput").ap()
    p2wt = nc.dram_tensor("p2wt", [PCH, NCH * 3 * C], BF, kind="ExternalInput").ap()
    w2t = nc.dram_tensor("w2t", [C, C], BF, kind="ExternalInput").ap()
    ident = nc.dram_tensor("ident", [C, C], BF, kind="ExternalInput").ap()
    out = nc.dram_tensor("out", [NS, C, HW], BF, kind="ExternalOutput").ap()

    with tile.TileContext(nc) as tc:
        with __import__("contextlib").ExitStack() as ctx:
            _body(
                tc, bass, mybir, xs, p2wt, w2t, ident, out, BF, F32, ctx,
                repeat, loop,
            )
    nc.compile()
    _CACHE[("nc", repeat, loop)] = nc
    return nc


def host_inputs(x, p2w, p5w, conv_w):
    """Shard + prep per-core input maps from full inputs."""
    x = np.asarray(x, dtype=np.float32).reshape(N, C, HW).astype(BF16NP)
    x = np.ascontiguousarray(x)
    p2w_ = (np.asarray(p2w, dtype=np.float32)[0] * SCALE).reshape(C, 3, HW)
    a = p2w_.transpose(2, 1, 0)                      # (p, d, c1)
    a = a.reshape(NCH, PCH, 3, C)                    # (j, pl, d, c1)
    a = np.ascontiguousarray(a.transpose(1, 0, 2, 3)).reshape(PCH, NCH * 3 * C)
    p2wt = a.astype(BF16NP)
    p5 = np.asarray(p5w, dtype=np.float32).reshape(C)
    cw = np.asarray(conv_w, dtype=np.float32)        # (C//G, C)
    W2 = cw[np.arange(C) % (C // G)] * p5[None, :]   # (c, c')
    w2t = np.ascontiguousarray(W2.T).astype(BF16NP)  # (c', c)
    ident = np.eye(C, dtype=BF16NP)
    in_maps = [
        {
            "xs": np.ascontiguousarray(x[i * NS : (i + 1) * NS]),
            "p2wt": p2wt,
            "w2t": w2t,
            "ident": ident,
        }
        for i in range(NCORES)
    ]
    return in_maps


def _get_runner(repeat=1, loop=1):
    """Build (once) a persistent jitted shard_map executable over 8 cores."""
    if ("runner", repeat, loop) in _CACHE:
        return _CACHE[("runner", repeat, loop)]
    import jax
    from jax.sharding import Mesh, PartitionSpec
    from jax.experimental.shard_map import shard_map
    from concourse import bass2jax, mybir

    nc = build(repeat, loop)
    bass2jax.install_neuronx_cc_hook()

    partition_name = nc.partition_id_tensor.name if nc.partition_id_tensor else None
    in_names, out_names, out_avals, zero_outs = [], [], [], []
    for alloc in nc.m.functions[0].allocations:
        if not isinstance(alloc, mybir.MemoryLocationSet):
            continue
        name = alloc.memorylocations[0].name
        if alloc.kind == "ExternalInput":
            if name != partition_name:
                in_names.append(name)
        elif alloc.kind == "ExternalOutput":
            shape = tuple(alloc.tensor_shape)
            dtype = mybir.dt.np(alloc.dtype)
            out_avals.append(jax.core.ShapedArray(shape, dtype))
            zero_outs.append(np.zeros(shape, dtype))
            out_names.append(name)
    n_params = len(in_names)
    n_outs = len(out_avals)
    all_in_names = list(in_names) + list(out_names)
    if partition_name is not None:
        all_in_names.append(partition_name)
    donate = tuple(range(n_params, n_params + n_outs))

    def _body(*args):
        operands = list(args)
        if partition_name is not None:
            operands.append(bass2jax.partition_id_tensor())
        outs = bass2jax._bass_exec_p.bind(
            *operands,
            out_avals=tuple(out_avals),
            in_names=tuple(all_in_names),
            out_names=tuple(out_names),
            lowering_input_output_aliases=(),
            sim_require_finite=True,
            sim_require_nnan=True,
            nc=nc,
        )
        return tuple(outs)

    devices = jax.devices()[:NCORES]
    mesh = Mesh(np.asarray(devices), ("core",))
    in_specs = (PartitionSpec("core"),) * (n_params + n_outs)
    out_specs = (PartitionSpec("core"),) * n_outs
    sharded = jax.jit(
        shard_map(
            _body, mesh=mesh, in_specs=in_specs, out_specs=out_specs, check_rep=False
        ),
        donate_argnums=donate,
        keep_unused=True,
    )
    runner = {
        "fn": sharded,
        "in_names": in_names,
        "out_names": out_names,
        "out_avals": out_avals,
        "mesh": mesh,
        "n_params": n_params,
    }
    _CACHE[("runner", repeat, loop)] = runner
    return runner


def _concat_inputs(runner, in_maps):
    return [
        np.concatenate([np.asarray(m[name]) for m in in_maps], axis=0)
        for name in runner["in_names"]
    ]


def _zero_bufs(runner):
    return [
        np.zeros((NCORES * a.shape[0], *a.shape[1:]), a.dtype)
        for a in runner["out_avals"]
    ]


def run_fast(in_maps):
    """Execute via the cached jitted executable; returns list of per-core dicts."""
    runner = _get_runner()
    out_arrs = runner["fn"](*_concat_inputs(runner, in_maps), *_zero_bufs(runner))
    res = []
    for c in range(NCORES):
        res.append(
            {
                name: np.asarray(out_arrs[i]).reshape(
                    NCORES, *runner["out_avals"][i].shape
                )[c]
                for i, name in enumerate(runner["out_names"])
            }
        )
    return res


def run(in_maps, trace=False, **kw):
    from concourse.bass_utils import run_bass_kernel_spmd

    nc = build()
    return run_bass_kernel_spmd(nc, in_maps, list(range(NCORES)), trace=trace, **kw)


def bench(in_maps, iters=30):
    """Pipelined timing of the device executable. Returns sec/iter."""
    import time

    import jax
    from jax.sharding import NamedSharding, PartitionSpec

    runner = _get_runner()
    sh = NamedSharding(runner["mesh"], PartitionSpec("core"))
    dev_in = [jax.device_put(a, sh) for a in _concat_inputs(runner, in_maps)]
    # donated output buffers are consumed per call: pre-stage iters copies
    zsets = [
        [jax.device_put(z, sh) for z in _zero_bufs(runner)] for _ in range(iters + 2)
    ]
    for z in zsets[0]:
        z.block_until_ready()
    # warmup
    out = runner["fn"](*dev_in, *zsets[-1])
    jax.block_until_ready(out)
    out = runner["fn"](*dev_in, *zsets[-2])
    jax.block_until_ready(out)
    t0 = time.perf_counter()
    outs = []
    for k in range(iters):
        outs.append(runner["fn"](*dev_in, *zsets[k]))
    jax.block_until_ready(outs)
    dt = (time.perf_counter() - t0) / iters
    return dt


def bench_repeat(in_maps, R=13, reps=12, iters=4, inner=4):
    """Per-kernel time from a hardware-looped program: loop=R vs loop=1,
    each with `inner` kernels unrolled in the loop body. The two programs
    are byte-identical except the loop trip count, so per-call dispatch
    overhead is identical and cancels in the difference; measurement
    bursts are interleaved so slow drift cancels too. Outputs are chained
    back in as donated buffers so no host->device upload is timed."""
    import time

    import jax
    from jax.sharding import NamedSharding, PartitionSpec

    state = {}
    for rep in (1, R):
        runner = _get_runner(inner, rep)
        sh = NamedSharding(runner["mesh"], PartitionSpec("core"))
        dev_in = [jax.device_put(a, sh) for a in _concat_inputs(runner, in_maps)]
        outs = [jax.device_put(z, sh) for z in _zero_bufs(runner)]
        jax.block_until_ready(dev_in)
        jax.block_until_ready(outs)
        outs = runner["fn"](*dev_in, *outs)  # warmup + first chain
        jax.block_until_ready(outs)
        state[rep] = [runner, dev_in, outs, []]

    for _ in range(reps):
        for rep in (1, R):
            runner, dev_in, outs, ts = state[rep]
            t0 = time.perf_counter()
            for _ in range(iters):
                outs = runner["fn"](*dev_in, *outs)
            jax.block_until_ready(outs)
            ts.append((time.perf_counter() - t0) / iters)
            state[rep][2] = outs

    t1s = sorted(state[1][3])
    tRs = sorted(state[R][3])
    k = max(3, len(t1s) // 2)
    t1 = sum(t1s[:k]) / k
    tR = sum(tRs[:k]) / k
    per = (tR - t1) / ((R - 1) * inner)
    return per, t1, tR


def kernel(x, p2w, p5w, conv_w):
    in_maps = host_inputs(x, p2w, p5w, conv_w)
    res = run_fast(in_maps)
    outs = [np.asarray(res[i]["out"]) for i in range(NCORES)]
    return (
        np.concatenate(outs, axis=0).reshape(N, C, H, W).astype(np.float32)
    )
